# revision 15
# baseline (speedup 1.0000x reference)
"""Capsule-routing kernel for trn2, 8-core data-parallel.

Math: reference computes u_hat = einsum('bid,dk->bik', u, W) reshaped to
[B, N=16, I, D=16], then 4 dynamic-routing iterations. u_hat (134 MB) is
never materialized; everything factors through u:

  iter 0: c uniform 1/16
  s[b,n,:]   = sum_i c[b,i,n] * u[b,i,:]           (PE f32r, contract i)
  o_full     = s @ W            [16, 256]           (diag blocks = o[b,n,:])
  o_hat      = l2norm(diag-masked o_full)
  V[b,:,n]   = W_n @ o_hat[b,n,:]  == W @ masked_o_hat^T
  logits     = u[b] @ V[b]      [I, 16]             (PE bf16, contract d)
  c          = softmax(logits, axis=n)              (no max-sub: |logit|<~4)
  final squash of diag(o_full) done on host.

Precision: s/o path runs float32r (single-pass PE); the logits path
(UT, V) runs bf16 — it only shapes the softmax weights. UT is
pre-transposed and cast on host, removing 128 PE transposes per core.
"""

import sys

for _p in ("/opt/trn_rl_repo", "/root/.axon_site/_ro/trn_rl_repo"):
    if _p not in sys.path:
        sys.path.insert(0, _p)

import numpy as np

B, I, D = 32, 4096, 128
NCAP, DCAP = 16, 16
K2 = NCAP * DCAP  # 256
N_CORES = 8
BPC = B // N_CORES  # batches per core
NT = I // 128  # 32 row tiles per batch
ROUTINGS = 4
EPS = 1e-7
L2_EPS = 1e-12

_CACHE = {}


def _split_multiwait_drains(nc, mybir):
    """walrus in this container rejects >1 sync-wait per instruction;
    hoist extra waits onto preceding same-engine NoOps."""
    for fn in nc.m.functions:
        for blk in fn.blocks:
            insts = list(blk.instructions)
            new_list = []
            changed = False
            for inst in insts:
                si = inst.sync_info
                if si is not None and si.on_wait and len(si.on_wait) > 1:
                    waits = list(si.on_wait)
                    for w in waits[:-1]:
                        nop = mybir.InstNoOp(
                            name=f"{inst.name}-wsplit-{w.id}", ins=[], outs=[]
                        )
                        nop.engine = inst.engine
                        nop.sync_info = mybir.SyncInfo(on_wait=[w], on_update=[])
                        new_list.append(nop)
                    inst.sync_info = mybir.SyncInfo(
                        on_wait=[waits[-1]], on_update=list(si.on_update)
                    )
                    changed = True
                new_list.append(inst)
            if changed:
                blk.instructions = new_list


def _build():
    import concourse.bass as bass
    import concourse.tile as tile
    import concourse.mybir as mybir

    f32 = mybir.dt.float32
    f32r = mybir.dt.float32r
    fp16 = mybir.dt.float16

    nc = bass.Bass()

    u = nc.dram_tensor("u", [BPC, I, D], fp16, kind="ExternalInput")
    ut_in = nc.dram_tensor("ut", [BPC, 128, NT, 128], fp16, kind="ExternalInput")
    w_in = nc.dram_tensor("w", [D, K2], fp16, kind="ExternalInput")
    wt2_in = nc.dram_tensor("wt2", [128, 2, 128], fp16, kind="ExternalInput")
    mask_in = nc.dram_tensor("mask", [NCAP, K2], f32, kind="ExternalInput")
    ident_in = nc.dram_tensor("ident", [16, 16], f32r, kind="ExternalInput")
    cbc_in = nc.dram_tensor("cbc", [1, NCAP], fp16, kind="ExternalInput")
    o_out = nc.dram_tensor("o_full", [BPC, NCAP, K2], f32, kind="ExternalOutput")

    with tile.TileContext(nc) as tc:
        with (
            tc.tile_pool(name="consts", bufs=1) as consts,
            tc.tile_pool(name="u_sb", bufs=BPC) as u_pool,
            tc.tile_pool(name="ut_sb", bufs=BPC) as ut_pool,
            tc.tile_pool(name="c_sb", bufs=6) as c_pool,
            tc.tile_pool(name="e_sb", bufs=3) as e_pool,
            tc.tile_pool(name="sm_sb", bufs=3) as sm_pool,
            tc.tile_pool(name="b_ps", bufs=3, space="PSUM") as b_pool,
            tc.tile_pool(name="sm_ps", bufs=2, space="PSUM") as smp_pool,
            tc.tile_pool(name="s_ps", bufs=2, space="PSUM") as sp_pool,
            tc.tile_pool(name="o_ps", bufs=1, space="PSUM") as op_pool,
        ):
            # ---- constants ----
            w_sb = consts.tile([128, K2], fp16)
            nc.sync.dma_start(out=w_sb, in_=w_in[:, :])
            wt2_sb = consts.tile([128, 2, 128], fp16)
            nc.sync.dma_start(out=wt2_sb, in_=wt2_in[:, :, :])
            mask_sb = consts.tile([NCAP, K2], f32)
            nc.sync.dma_start(out=mask_sb, in_=mask_in[:, :])
            ident = consts.tile([16, 16], f32r)
            nc.sync.dma_start(out=ident, in_=ident_in[:, :])
            cbc = consts.tile([1, NCAP], fp16)
            nc.sync.dma_start(out=cbc, in_=cbc_in[:, :])

            # ---- batch-major: load + route each batch; Tile overlaps ----
            u_tiles = []
            ut_tiles = []
            for b in range(BPC):
                utb = ut_pool.tile([128, NT, 128], fp16, tag="ut")
                for c in range(2):
                    nc.sync.dma_start(
                        out=utb[:, 16 * c : 16 * (c + 1), :],
                        in_=ut_in[b, :, 16 * c : 16 * (c + 1), :],
                    )
                ut_tiles.append(utb)
                ub = u_pool.tile([128, NT, 128], fp16, tag="u")
                src = u[b, :, :].rearrange("(t p) d -> p t d", p=128)
                for c in range(2):
                    nc.sync.dma_start(
                        out=ub[:, 16 * c : 16 * (c + 1), :],
                        in_=src[:, 16 * c : 16 * (c + 1), :],
                    )
                u_tiles.append(ub)

            # ---- routing ----
            c_cur = [None for _ in range(BPC)]
            for b in range(BPC):
                for rt in range(ROUTINGS):
                    ub, utb = u_tiles[b], ut_tiles[b]
                    o_ps = op_pool.tile([NCAP, K2], f32, tag="o_ps")
                    if rt == 0:
                        # uniform c: o = (1/16) colsum(u) @ W, same row per n
                        colsum = sm_pool.tile([128, 1], fp16, tag="colsum")
                        with nc.allow_low_precision(
                            reason="colsum feeds fp16 matmul; |err| ~2e-4 rel"
                        ):
                            nc.vector.reduce_sum(
                                out=colsum,
                                in_=utb.rearrange("p t d -> p (t d)"),
                                axis=mybir.AxisListType.X,
                            )
                        o1_ps = smp_pool.tile([1, K2], f32, tag="sm")
                        nc.tensor.matmul(
                            o1_ps, lhsT=colsum, rhs=w_sb, start=True, stop=True
                        )
                        o1_sb = sm_pool.tile([1, K2], fp16, tag="o1_sb")
                        nc.vector.tensor_copy(out=o1_sb, in_=o1_ps)
                        # broadcast to 16 rows with 1/16 scale folded in
                        nc.tensor.matmul(
                            o_ps, lhsT=cbc, rhs=o1_sb, start=True, stop=True
                        )
                    else:
                        # ST[dd, n] = sum_t U_t^T @ C_t  (fp16)
                        s_ps = sp_pool.tile([128, NCAP], f32, tag="s_ps")
                        for t in range(NT):
                            nc.tensor.matmul(
                                s_ps,
                                lhsT=ub[:, t, :],
                                rhs=c_cur[b][:, t, :],
                                start=(t == 0),
                                stop=(t == NT - 1),
                            )
                        st_sb = sm_pool.tile([128, NCAP], fp16, tag="st_sb")
                        nc.vector.tensor_copy(out=st_sb, in_=s_ps)
                        # o_full = s @ W  [16, 256]
                        nc.tensor.matmul(
                            o_ps, lhsT=st_sb, rhs=w_sb, start=True, stop=True
                        )

                    if rt == ROUTINGS - 1:
                        om = sm_pool.tile([NCAP, K2], f32, tag="om")
                        nc.vector.tensor_copy(out=om, in_=o_ps)
                        nc.sync.dma_start(out=o_out[b, :, :], in_=om)
                        continue

                    # l2-normalize diag-masked o (DVE/ACT, [16,*] rows)
                    om = sm_pool.tile([NCAP, K2], f32r, tag="om")
                    nc.vector.tensor_mul(om, o_ps, mask_sb)
                    sq = sm_pool.tile([NCAP, K2], f32, tag="sq")
                    ss = sm_pool.tile([NCAP, 1], f32, tag="ss")
                    nc.scalar.activation(
                        out=sq,
                        in_=om,
                        func=mybir.ActivationFunctionType.Square,
                        accum_out=ss,
                    )
                    nc.vector.tensor_scalar_max(out=ss, in0=ss, scalar1=L2_EPS)
                    sr = sm_pool.tile([NCAP, 1], f32, tag="sr")
                    nc.scalar.activation(
                        out=sr, in_=ss, func=mybir.ActivationFunctionType.Sqrt
                    )
                    rn = sm_pool.tile([NCAP, 1], f32, tag="rn")
                    nc.vector.reciprocal(out=rn, in_=sr)
                    oh = sm_pool.tile([NCAP, K2], f32r, tag="oh")
                    nc.vector.tensor_scalar_mul(out=oh, in0=om, scalar1=rn)

                    # V = W @ masked_o_hat^T : [128, 16]  (bf16 path)
                    oht_sb = sm_pool.tile([128, 2, NCAP], fp16, tag="oht")
                    for h in range(2):
                        oht_ps = smp_pool.tile([128, NCAP], f32r, tag="sm")
                        nc.tensor.transpose(
                            oht_ps, oh[:, 128 * h : 128 * (h + 1)], ident
                        )
                        nc.vector.tensor_copy(out=oht_sb[:, h, :], in_=oht_ps)
                    v_ps = smp_pool.tile([128, NCAP], f32, tag="sm")
                    for h in range(2):
                        nc.tensor.matmul(
                            v_ps,
                            lhsT=wt2_sb[:, h, :],
                            rhs=oht_sb[:, h, :],
                            start=(h == 0),
                            stop=(h == 1),
                        )
                    v_sb = sm_pool.tile([128, NCAP], fp16, tag="v_sb")
                    nc.vector.tensor_copy(out=v_sb, in_=v_ps)

                    # logits = u @ V : [128, NT, 16] (bf16); softmax over n
                    b_ps = b_pool.tile([128, NT, NCAP], f32, tag="bps")
                    for t in range(NT):
                        nc.tensor.matmul(
                            b_ps[:, t, :],
                            lhsT=utb[:, t, :],
                            rhs=v_sb,
                            start=True,
                            stop=True,
                        )
                    e_sb = e_pool.tile([128, NT, NCAP], f32, tag="e")
                    nc.scalar.activation(
                        out=e_sb, in_=b_ps, func=mybir.ActivationFunctionType.Exp
                    )
                    ssum = sm_pool.tile([128, NT], f32, tag="ssum")
                    nc.vector.reduce_sum(out=ssum, in_=e_sb, axis=mybir.AxisListType.X)
                    rs = sm_pool.tile([128, NT], f32, tag="rs")
                    nc.vector.reciprocal(out=rs, in_=ssum)
                    c_new = c_pool.tile([128, NT, NCAP], fp16, tag="c")
                    nc.vector.tensor_mul(
                        c_new, e_sb, rs[:, :].to_broadcast((128, NT, NCAP))
                    )
                    c_cur[b] = c_new

    _split_multiwait_drains(nc, mybir)
    return nc


def _consts():
    ident = np.eye(16, dtype=np.float32)
    mask = np.zeros((NCAP, K2), dtype=np.float32)
    for n in range(NCAP):
        mask[n, n * DCAP : (n + 1) * DCAP] = 1.0
    return ident, mask


def _prep_inputs(u_vecs, W):
    import ml_dtypes

    u_vecs = np.ascontiguousarray(u_vecs, dtype=np.float32)
    W = np.ascontiguousarray(W, dtype=np.float32)
    u16 = u_vecs.astype(np.float16)
    wt2 = np.ascontiguousarray(
        W.T.reshape(2, 128, 128).transpose(1, 0, 2), dtype=np.float32
    ).astype(np.float16)  # wt2[kk, h, dd] = W[dd, h*128+kk]
    ident, mask = _consts()
    cbc = np.full((1, NCAP), 1.0 / NCAP, dtype=np.float16)
    # ut[b, dd, t, ii] = u[b, t*128+ii, dd], fp16
    ut_all = np.ascontiguousarray(
        u16.reshape(B, NT, 128, D).transpose(0, 3, 1, 2)
    )

    in_maps = []
    for c in range(N_CORES):
        in_maps.append(
            {
                "u": np.ascontiguousarray(u16[c * BPC : (c + 1) * BPC]),
                "ut": np.ascontiguousarray(ut_all[c * BPC : (c + 1) * BPC]),
                "w": W.astype(np.float16),
                "wt2": wt2,
                "mask": mask,
                "ident": ident,
                "cbc": cbc,
            }
        )
    return in_maps


def kernel(u_vecs: np.ndarray, W: np.ndarray) -> np.ndarray:
    from concourse.bass_utils import run_bass_kernel_spmd

    if "nc" not in _CACHE:
        _CACHE["nc"] = _build()
    nc = _CACHE["nc"]

    in_maps = _prep_inputs(u_vecs, W)
    res = run_bass_kernel_spmd(nc, in_maps, core_ids=list(range(N_CORES)))

    outs = []
    idx = np.arange(NCAP)
    for c in range(N_CORES):
        of = res.results[c]["o_full"]  # [BPC, 16, 256]
        o = of.reshape(BPC, NCAP, NCAP, DCAP)[:, idx, idx, :]  # diag blocks
        outs.append(o)
    o = np.concatenate(outs, axis=0).astype(np.float32)  # [B, 16, 16]

    # squash (host, matches reference)
    s = np.sum(np.square(o), axis=-1, keepdims=True) + EPS
    scale = np.sqrt(s) / (0.5 + s)
    return (scale * o).astype(np.float32)


# revision 16
# speedup vs baseline: 1.5625x; 1.5625x over previous
"""Capsule-routing kernel for trn2, 8-core data-parallel.

Math: reference computes u_hat = einsum('bid,dk->bik', u, W) reshaped to
[B, N=16, I, D=16], then 4 dynamic-routing iterations. u_hat (134 MB) is
never materialized; everything factors through u:

  iter 0: c uniform 1/16
  s[b,n,:]   = sum_i c[b,i,n] * u[b,i,:]           (PE f32r, contract i)
  o_full     = s @ W            [16, 256]           (diag blocks = o[b,n,:])
  o_hat      = l2norm(diag-masked o_full)
  V[b,:,n]   = W_n @ o_hat[b,n,:]  == W @ masked_o_hat^T
  logits     = u[b] @ V[b]      [I, 16]             (PE bf16, contract d)
  c          = softmax(logits, axis=n)              (no max-sub: |logit|<~4)
  final squash of diag(o_full) done on host.

Precision: s/o path runs float32r (single-pass PE); the logits path
(UT, V) runs bf16 — it only shapes the softmax weights. UT is
pre-transposed and cast on host, removing 128 PE transposes per core.
"""

import sys

for _p in ("/opt/trn_rl_repo", "/root/.axon_site/_ro/trn_rl_repo"):
    if _p not in sys.path:
        sys.path.insert(0, _p)

import numpy as np

B, I, D = 32, 4096, 128
NCAP, DCAP = 16, 16
K2 = NCAP * DCAP  # 256
N_CORES = 8
BPC = B // N_CORES  # batches per core
NT = I // 128  # 32 row tiles per batch
ROUTINGS = 4
EPS = 1e-7
L2_EPS = 1e-12

_CACHE = {}


def _split_multiwait_drains(nc, mybir):
    """walrus in this container rejects >1 sync-wait per instruction;
    hoist extra waits onto preceding same-engine NoOps."""
    for fn in nc.m.functions:
        for blk in fn.blocks:
            insts = list(blk.instructions)
            new_list = []
            changed = False
            for inst in insts:
                si = inst.sync_info
                if si is not None and si.on_wait and len(si.on_wait) > 1:
                    waits = list(si.on_wait)
                    for w in waits[:-1]:
                        nop = mybir.InstNoOp(
                            name=f"{inst.name}-wsplit-{w.id}", ins=[], outs=[]
                        )
                        nop.engine = inst.engine
                        nop.sync_info = mybir.SyncInfo(on_wait=[w], on_update=[])
                        new_list.append(nop)
                    inst.sync_info = mybir.SyncInfo(
                        on_wait=[waits[-1]], on_update=list(si.on_update)
                    )
                    changed = True
                new_list.append(inst)
            if changed:
                blk.instructions = new_list


def _build():
    import concourse.bass as bass
    import concourse.tile as tile
    import concourse.mybir as mybir

    f32 = mybir.dt.float32
    f32r = mybir.dt.float32r
    fp16 = mybir.dt.float16

    nc = bass.Bass()

    u = nc.dram_tensor("u", [BPC, I, D], fp16, kind="ExternalInput")
    ut_in = nc.dram_tensor("ut", [BPC, 128, NT, 128], fp16, kind="ExternalInput")
    w_in = nc.dram_tensor("w", [D, K2], fp16, kind="ExternalInput")
    wt2_in = nc.dram_tensor("wt2", [128, 2, 128], fp16, kind="ExternalInput")
    mask_in = nc.dram_tensor("mask", [NCAP, K2], f32, kind="ExternalInput")
    ident_in = nc.dram_tensor("ident", [16, 16], f32r, kind="ExternalInput")
    cbc_in = nc.dram_tensor("cbc", [1, NCAP], fp16, kind="ExternalInput")
    o_out = nc.dram_tensor("o_full", [BPC, NCAP, K2], f32, kind="ExternalOutput")

    with tile.TileContext(nc) as tc:
        with (
            tc.tile_pool(name="consts", bufs=1) as consts,
            tc.tile_pool(name="u_sb", bufs=BPC) as u_pool,
            tc.tile_pool(name="ut_sb", bufs=BPC) as ut_pool,
            tc.tile_pool(name="c_sb", bufs=6) as c_pool,
            tc.tile_pool(name="e_sb", bufs=3) as e_pool,
            tc.tile_pool(name="sm_sb", bufs=3) as sm_pool,
            tc.tile_pool(name="b_ps", bufs=3, space="PSUM") as b_pool,
            tc.tile_pool(name="sm_ps", bufs=2, space="PSUM") as smp_pool,
            tc.tile_pool(name="s_ps", bufs=2, space="PSUM") as sp_pool,
            tc.tile_pool(name="o_ps", bufs=1, space="PSUM") as op_pool,
        ):
            # ---- constants ----
            w_sb = consts.tile([128, K2], fp16)
            nc.sync.dma_start(out=w_sb, in_=w_in[:, :])
            wt2_sb = consts.tile([128, 2, 128], fp16)
            nc.sync.dma_start(out=wt2_sb, in_=wt2_in[:, :, :])
            mask_sb = consts.tile([NCAP, K2], f32)
            nc.sync.dma_start(out=mask_sb, in_=mask_in[:, :])
            ident = consts.tile([16, 16], f32r)
            nc.sync.dma_start(out=ident, in_=ident_in[:, :])
            cbc = consts.tile([1, NCAP], fp16)
            nc.sync.dma_start(out=cbc, in_=cbc_in[:, :])

            # ---- batch-major: load + route each batch; Tile overlaps ----
            u_tiles = []
            ut_tiles = []
            for b in range(BPC):
                utb = ut_pool.tile([128, NT, 128], fp16, tag="ut")
                for c in range(2):
                    nc.sync.dma_start(
                        out=utb[:, 16 * c : 16 * (c + 1), :],
                        in_=ut_in[b, :, 16 * c : 16 * (c + 1), :],
                    )
                ut_tiles.append(utb)
                ub = u_pool.tile([128, NT, 128], fp16, tag="u")
                src = u[b, :, :].rearrange("(t p) d -> p t d", p=128)
                for c in range(2):
                    nc.sync.dma_start(
                        out=ub[:, 16 * c : 16 * (c + 1), :],
                        in_=src[:, 16 * c : 16 * (c + 1), :],
                    )
                u_tiles.append(ub)

            # ---- routing ----
            c_cur = [None for _ in range(BPC)]
            for rt in range(ROUTINGS):
                for b in range(BPC):
                    ub, utb = u_tiles[b], ut_tiles[b]
                    o_ps = op_pool.tile([NCAP, K2], f32, tag="o_ps")
                    if rt == 0:
                        # uniform c: o = (1/16) colsum(u) @ W, same row per n
                        colsum = sm_pool.tile([128, 1], fp16, tag="colsum")
                        with nc.allow_low_precision(
                            reason="colsum feeds fp16 matmul; |err| ~2e-4 rel"
                        ):
                            nc.vector.reduce_sum(
                                out=colsum,
                                in_=utb.rearrange("p t d -> p (t d)"),
                                axis=mybir.AxisListType.X,
                            )
                        o1_ps = smp_pool.tile([1, K2], f32, tag="sm")
                        nc.tensor.matmul(
                            o1_ps, lhsT=colsum, rhs=w_sb, start=True, stop=True
                        )
                        o1_sb = sm_pool.tile([1, K2], fp16, tag="o1_sb")
                        nc.vector.tensor_copy(out=o1_sb, in_=o1_ps)
                        # broadcast to 16 rows with 1/16 scale folded in
                        nc.tensor.matmul(
                            o_ps, lhsT=cbc, rhs=o1_sb, start=True, stop=True
                        )
                    else:
                        # ST[dd, n] = sum_t U_t^T @ C_t  (fp16)
                        s_ps = sp_pool.tile([128, NCAP], f32, tag="s_ps")
                        for t in range(NT):
                            nc.tensor.matmul(
                                s_ps,
                                lhsT=ub[:, t, :],
                                rhs=c_cur[b][:, t, :],
                                start=(t == 0),
                                stop=(t == NT - 1),
                            )
                        st_sb = sm_pool.tile([128, NCAP], fp16, tag="st_sb")
                        nc.vector.tensor_copy(out=st_sb, in_=s_ps)
                        # o_full = s @ W  [16, 256]
                        nc.tensor.matmul(
                            o_ps, lhsT=st_sb, rhs=w_sb, start=True, stop=True
                        )

                    if rt == ROUTINGS - 1:
                        om = sm_pool.tile([NCAP, K2], f32, tag="om")
                        nc.vector.tensor_copy(out=om, in_=o_ps)
                        nc.sync.dma_start(out=o_out[b, :, :], in_=om)
                        continue

                    # l2-normalize diag-masked o (DVE/ACT, [16,*] rows)
                    om = sm_pool.tile([NCAP, K2], f32r, tag="om")
                    nc.vector.tensor_mul(om, o_ps, mask_sb)
                    sq = sm_pool.tile([NCAP, K2], f32, tag="sq")
                    ss = sm_pool.tile([NCAP, 1], f32, tag="ss")
                    nc.scalar.activation(
                        out=sq,
                        in_=om,
                        func=mybir.ActivationFunctionType.Square,
                        accum_out=ss,
                    )
                    nc.vector.tensor_scalar_max(out=ss, in0=ss, scalar1=L2_EPS)
                    sr = sm_pool.tile([NCAP, 1], f32, tag="sr")
                    nc.scalar.activation(
                        out=sr, in_=ss, func=mybir.ActivationFunctionType.Sqrt
                    )
                    rn = sm_pool.tile([NCAP, 1], f32, tag="rn")
                    nc.vector.reciprocal(out=rn, in_=sr)
                    oh = sm_pool.tile([NCAP, K2], f32r, tag="oh")
                    nc.vector.tensor_scalar_mul(out=oh, in0=om, scalar1=rn)

                    # V = W @ masked_o_hat^T : [128, 16]  (bf16 path)
                    oht_sb = sm_pool.tile([128, 2, NCAP], fp16, tag="oht")
                    for h in range(2):
                        oht_ps = smp_pool.tile([128, NCAP], f32r, tag="sm")
                        nc.tensor.transpose(
                            oht_ps, oh[:, 128 * h : 128 * (h + 1)], ident
                        )
                        nc.vector.tensor_copy(out=oht_sb[:, h, :], in_=oht_ps)
                    v_ps = smp_pool.tile([128, NCAP], f32, tag="sm")
                    for h in range(2):
                        nc.tensor.matmul(
                            v_ps,
                            lhsT=wt2_sb[:, h, :],
                            rhs=oht_sb[:, h, :],
                            start=(h == 0),
                            stop=(h == 1),
                        )
                    v_sb = sm_pool.tile([128, NCAP], fp16, tag="v_sb")
                    nc.vector.tensor_copy(out=v_sb, in_=v_ps)

                    # logits = u @ V : [128, NT, 16] (bf16); softmax over n
                    b_ps = b_pool.tile([128, NT, NCAP], f32, tag="bps")
                    for t in range(NT):
                        nc.tensor.matmul(
                            b_ps[:, t, :],
                            lhsT=utb[:, t, :],
                            rhs=v_sb,
                            start=True,
                            stop=True,
                        )
                    e_sb = e_pool.tile([128, NT, NCAP], f32, tag="e")
                    nc.scalar.activation(
                        out=e_sb, in_=b_ps, func=mybir.ActivationFunctionType.Exp
                    )
                    ssum = sm_pool.tile([128, NT], f32, tag="ssum")
                    nc.vector.reduce_sum(out=ssum, in_=e_sb, axis=mybir.AxisListType.X)
                    rs = sm_pool.tile([128, NT], f32, tag="rs")
                    nc.vector.reciprocal(out=rs, in_=ssum)
                    c_new = c_pool.tile([128, NT, NCAP], fp16, tag="c")
                    nc.vector.tensor_mul(
                        c_new, e_sb, rs[:, :].to_broadcast((128, NT, NCAP))
                    )
                    c_cur[b] = c_new

    _split_multiwait_drains(nc, mybir)
    return nc


def _consts():
    ident = np.eye(16, dtype=np.float32)
    mask = np.zeros((NCAP, K2), dtype=np.float32)
    for n in range(NCAP):
        mask[n, n * DCAP : (n + 1) * DCAP] = 1.0
    return ident, mask


def _prep_inputs(u_vecs, W):
    import ml_dtypes

    u_vecs = np.ascontiguousarray(u_vecs, dtype=np.float32)
    W = np.ascontiguousarray(W, dtype=np.float32)
    u16 = u_vecs.astype(np.float16)
    wt2 = np.ascontiguousarray(
        W.T.reshape(2, 128, 128).transpose(1, 0, 2), dtype=np.float32
    ).astype(np.float16)  # wt2[kk, h, dd] = W[dd, h*128+kk]
    ident, mask = _consts()
    cbc = np.full((1, NCAP), 1.0 / NCAP, dtype=np.float16)
    # ut[b, dd, t, ii] = u[b, t*128+ii, dd], fp16
    ut_all = np.ascontiguousarray(
        u16.reshape(B, NT, 128, D).transpose(0, 3, 1, 2)
    )

    in_maps = []
    for c in range(N_CORES):
        in_maps.append(
            {
                "u": np.ascontiguousarray(u16[c * BPC : (c + 1) * BPC]),
                "ut": np.ascontiguousarray(ut_all[c * BPC : (c + 1) * BPC]),
                "w": W.astype(np.float16),
                "wt2": wt2,
                "mask": mask,
                "ident": ident,
                "cbc": cbc,
            }
        )
    return in_maps


def kernel(u_vecs: np.ndarray, W: np.ndarray) -> np.ndarray:
    from concourse.bass_utils import run_bass_kernel_spmd

    if "nc" not in _CACHE:
        _CACHE["nc"] = _build()
    nc = _CACHE["nc"]

    in_maps = _prep_inputs(u_vecs, W)
    res = run_bass_kernel_spmd(nc, in_maps, core_ids=list(range(N_CORES)))

    outs = []
    idx = np.arange(NCAP)
    for c in range(N_CORES):
        of = res.results[c]["o_full"]  # [BPC, 16, 256]
        o = of.reshape(BPC, NCAP, NCAP, DCAP)[:, idx, idx, :]  # diag blocks
        outs.append(o)
    o = np.concatenate(outs, axis=0).astype(np.float32)  # [B, 16, 16]

    # squash (host, matches reference)
    s = np.sum(np.square(o), axis=-1, keepdims=True) + EPS
    scale = np.sqrt(s) / (0.5 + s)
    return (scale * o).astype(np.float32)
